# revision 63
# baseline (speedup 1.0000x reference)
"""MLA attention (DeepSeek-style, LoRA Q/KV) on 8 Trainium2 NeuronCores.

All matmuls in bf16 (fp8 fails the 2e-2 absmax gate: quantization noise of
a random matmul does not average down relative to its output).

Two SPMD launches (host gathers between them):
  L1 (sequence-parallel, 256 tokens/core): LoRA-A projections. comp is
  rms-normalized on-chip (its scale is ready early); the q-path ships RAW
  with per-token sq-sums, and the host computes rq = rsqrt(mean+eps) for
  free between launches (kills the L1 tail).
  L2 (tensor-parallel, 2 heads/core): q/k/v LoRA-B projections (rq folded
  into the PSUM-drain copies), scores^T per key tile, exp(bf16) pairs,
  attn@v in PSUM, softmax denominator via a DVE add-tree over the exp
  pairs plus tiny ones-matmuls (frees PSUM banks for triple-buffered
  scores), per-head normalize, output projection. Host sums 8 partials.

Schedule: scores get 3 PSUM buffers (av/den accumulate one 512-query psl
at a time, 1 bank each; psl1 replays the kept exp tiles as a phase B
interleaved into the next pass). Late q-tiles, v-tiles and the first
output projection ride the attention tp-loops as fillers so the in-order
PE stream never drains. All DMAs use partition-major host-packed layouts
(long contiguous descriptors), big ones on the SP queue in consumption
order, tiny ones on the ACT queue.
"""

import math
from contextlib import ExitStack

import numpy as np
import ml_dtypes

import concourse.bass as bass
import concourse.mybir as mybir
import concourse.tile as tile
from concourse import bacc
from concourse.bass_utils import run_bass_kernel_spmd

BF = ml_dtypes.bfloat16
F32 = mybir.dt.float32
BF16 = mybir.dt.bfloat16
AF = mybir.ActivationFunctionType
ALU = mybir.AluOpType

D_MODEL = 2048
NH = 16
Q_LORA = 1536
KV_LORA = 512
ROPE = 64
NOPE = 128
VDIM = 128
QHD = NOPE + ROPE  # 192
SEQ = 2048
N_CORES = 8
S_LOC = SEQ // N_CORES  # 256 tokens per core in L1
EPS = 1e-6
SCALE = 1.0 / math.sqrt(128.0)  # 1/sqrt(HEAD_DIM), as in the reference

_CACHE = {}


def _perm_rope_T(n):
    """lhsT for P @ v where (P@v)[2i] = -v[2i+1], (P@v)[2i+1] = v[2i]."""
    P = np.zeros((n, n), np.float32)
    for i in range(n // 2):
        P[2 * i, 2 * i + 1] = -1.0
        P[2 * i + 1, 2 * i] = 1.0
    return np.ascontiguousarray(P.T).astype(BF)


def _pack(w, k):
    """[k*128, n] -> partition-major [128, k, n] (one long row per partition)."""
    n = w.shape[-1]
    return np.ascontiguousarray(w.reshape(k, 128, n).transpose(1, 0, 2))


# --------------------------------------------------------------------------
# Launch 1: sequence-sharded LoRA-A projections (+ comp norm, q sq-sums)
# --------------------------------------------------------------------------

def build_l1():
    nc = bacc.Bacc("TRN2", target_bir_lowering=False, debug=False,
                   enable_asserts=True, num_devices=N_CORES)
    KD = D_MODEL // 128   # 16
    MQ = Q_LORA // 128    # 12

    xT = nc.dram_tensor("xT", [128, KD, S_LOC], BF16, kind="ExternalInput").ap()
    Wqa = nc.dram_tensor("Wqa", [128, KD, Q_LORA], BF16, kind="ExternalInput").ap()
    Wkva = nc.dram_tensor("Wkva", [128, KD, 576], BF16, kind="ExternalInput").ap()
    cosT = nc.dram_tensor("cosT", [ROPE, S_LOC], F32, kind="ExternalInput").ap()
    sinT = nc.dram_tensor("sinT", [ROPE, S_LOC], F32, kind="ExternalInput").ap()
    permT = nc.dram_tensor("permT", [ROPE, ROPE], BF16, kind="ExternalInput").ap()
    ones = nc.dram_tensor("ones", [128, 1], BF16, kind="ExternalInput").ap()

    tnT = nc.dram_tensor("tnT", [128, MQ, S_LOC], BF16, kind="ExternalOutput").ap()
    compT = nc.dram_tensor("compT", [128, 4, S_LOC], BF16,
                           kind="ExternalOutput").ap()
    kpeT = nc.dram_tensor("kpeT", [ROPE, S_LOC], BF16, kind="ExternalOutput").ap()
    sqq = nc.dram_tensor("sqq", [1, S_LOC], F32, kind="ExternalOutput").ap()

    with tile.TileContext(nc) as tc, ExitStack() as ctx:
        const = ctx.enter_context(tc.tile_pool(name="const", bufs=1))
        big = ctx.enter_context(tc.tile_pool(name="big", bufs=1))
        work = ctx.enter_context(tc.tile_pool(name="work", bufs=3))
        ps = ctx.enter_context(tc.tile_pool(name="ps", bufs=3, space="PSUM"))
        ps1 = ctx.enter_context(tc.tile_pool(name="ps1", bufs=1, space="PSUM"))

        # big DMAs first on SP queue; tiny consts ride the ACT queue
        sb_xT = big.tile([128, KD, S_LOC], BF16, tag="xT")
        sb_wkva = big.tile([128, KD, 576], BF16, tag="wkva")
        for k0, k1 in ((0, 4), (4, 8), (8, KD)):
            nc.sync.dma_start(sb_xT[:, k0:k1, :], xT[:, k0:k1, :])
            nc.sync.dma_start(sb_wkva[:, k0:k1, :], Wkva[:, k0:k1, :])
        sb_wqa = big.tile([128, KD, Q_LORA], BF16, tag="wqa")
        for nchunk in range(3):  # 512-col chunks so q m-tiles start early
            nsl = slice(nchunk * 512, (nchunk + 1) * 512)
            nc.sync.dma_start(sb_wqa[:, :, nsl], Wqa[:, :, nsl])
        sb_cos = const.tile([ROPE, S_LOC], F32, tag="cos")
        nc.scalar.dma_start(sb_cos[:], cosT)
        sb_sin = const.tile([ROPE, S_LOC], F32, tag="sin")
        nc.scalar.dma_start(sb_sin[:], sinT)
        sb_perm = const.tile([ROPE, ROPE], BF16, tag="perm")
        nc.scalar.dma_start(sb_perm[:], permT)
        sb_ones = const.tile([128, 1], BF16, tag="ones")
        nc.scalar.dma_start(sb_ones[:], ones)

        # ---- ckv.T: 4 comp m-tiles + k_pe [64]
        # kv norm applied HERE (its scale is ready early, no tail impact)
        c_raw = big.tile([128, 4, S_LOC], BF16, tag="craw")
        c_sq = big.tile([128, 4, S_LOC], BF16, tag="csq")
        for m in range(4):
            acc = ps.tile([128, S_LOC], F32, tag="acc")
            for k in range(KD):
                nc.tensor.matmul(acc[:], sb_wkva[:, k, m * 128:(m + 1) * 128],
                                 sb_xT[:, k, :],
                                 start=(k == 0), stop=(k == KD - 1))
            nc.scalar.copy(c_raw[:, m, :], acc[:])
            nc.vector.tensor_mul(c_sq[:, m, :], c_raw[:, m, :], c_raw[:, m, :])

        # k_pe rows 512:576 -> [64, S]; rope it (k_pe is not normalized)
        kpe_acc = ps1.tile([64, S_LOC], F32, tag="kpe")
        for k in range(KD):
            nc.tensor.matmul(kpe_acc[:], sb_wkva[:, k, 512:576],
                             sb_xT[:, k, :],
                             start=(k == 0), stop=(k == KD - 1))
        kpe_sb = work.tile([64, S_LOC], BF16, tag="kpesb")
        nc.scalar.copy(kpe_sb[:], kpe_acc[:])
        swap_ps = ps1.tile([64, S_LOC], F32, tag="swap")
        nc.tensor.matmul(swap_ps[:], sb_perm[:], kpe_sb[:], start=True, stop=True)
        kc = work.tile([64, S_LOC], F32, tag="kc")
        nc.vector.tensor_mul(kc[:], kpe_sb[:], sb_cos[:])
        ks = work.tile([64, S_LOC], F32, tag="ks")
        nc.vector.tensor_mul(ks[:], swap_ps[:], sb_sin[:])
        kout = work.tile([64, S_LOC], BF16, tag="kout")
        nc.vector.tensor_add(kout[:], kc[:], ks[:])
        nc.sync.dma_start(kpeT, kout[:])

        # kv rms scale (partition sum via ones-matmul) and apply
        sqkv_ps = ps1.tile([1, S_LOC], F32, tag="sqkv")
        for m in range(4):
            nc.tensor.matmul(sqkv_ps[:], sb_ones[:], c_sq[:, m, :],
                             start=(m == 0), stop=(m == 3))
        eps_t = const.tile([1, 1], F32, tag="eps")
        nc.vector.memset(eps_t[:], EPS)
        sroot = work.tile([1, S_LOC], F32, tag="sroot")
        nc.scalar.activation(sroot[:], sqkv_ps[:], AF.Sqrt,
                             bias=eps_t[:], scale=1.0 / KV_LORA)
        rec = work.tile([1, S_LOC], F32, tag="rec")
        nc.vector.reciprocal(rec[:], sroot[:])
        rkv_b = work.tile([128, S_LOC], F32, tag="rkvb")
        nc.gpsimd.partition_broadcast(rkv_b[:], rec[:])
        o_cn = big.tile([128, 4, S_LOC], BF16, tag="ocn")
        for m in range(4):
            nc.vector.tensor_mul(o_cn[:, m, :], c_raw[:, m, :], rkv_b[:])
        nc.sync.dma_start(compT, o_cn[:])

        # ---- t.T: 12 m-tiles of [128, 256]; RAW bf16 out + sq-sums
        # (sq partition-sums accumulate per-m so the tail is one matmul)
        o_tn = big.tile([128, MQ, S_LOC], BF16, tag="otn")
        t_sq = big.tile([128, MQ, S_LOC], BF16, tag="tsq")
        sqq_ps = ps1.tile([1, S_LOC], F32, tag="sqq")
        for m in range(MQ):
            acc = ps.tile([128, S_LOC], F32, tag="acc")
            for k in range(KD):
                nc.tensor.matmul(acc[:], sb_wqa[:, k, m * 128:(m + 1) * 128],
                                 sb_xT[:, k, :],
                                 start=(k == 0), stop=(k == KD - 1))
            nc.scalar.copy(o_tn[:, m, :], acc[:])
            nc.vector.tensor_mul(t_sq[:, m, :], o_tn[:, m, :], o_tn[:, m, :])
            nc.tensor.matmul(sqq_ps[:], sb_ones[:], t_sq[:, m, :],
                             start=(m == 0), stop=(m == MQ - 1))
            if m % 2 == 1:  # write in 2-m-tile chunks to overlap DMA
                nc.sync.dma_start(tnT[:, m - 1:m + 1, :],
                                  o_tn[:, m - 1:m + 1, :])

        sqq_sb = work.tile([1, S_LOC], F32, tag="sqqsb")
        nc.scalar.copy(sqq_sb[:], sqq_ps[:])
        nc.sync.dma_start(sqq, sqq_sb[:])

    nc.compile()
    return nc


# --------------------------------------------------------------------------
# Launch 2: head-sharded attention (2 heads per core)
# --------------------------------------------------------------------------

def build_l2():
    nc = bacc.Bacc("TRN2", target_bir_lowering=False, debug=False,
                   enable_asserts=True, num_devices=N_CORES)
    KQ = Q_LORA // 128    # 12
    KKV = KV_LORA // 128  # 4
    ST = SEQ // 128       # 16 key tiles
    SB = 1024             # query block
    NSB = SEQ // SB       # 2

    tnT = nc.dram_tensor("tnT", [128, KQ, SEQ], BF16, kind="ExternalInput").ap()
    compT = nc.dram_tensor("compT", [128, KKV, SEQ], BF16,
                           kind="ExternalInput").ap()
    kpeT = nc.dram_tensor("kpeT", [ROPE, SEQ], BF16, kind="ExternalInput").ap()
    # Wqb cols reordered [h0 nope | h1 nope | h0 rope | h1 rope], qln folded
    Wqb = nc.dram_tensor("Wqb", [128, KQ, 2 * QHD], BF16,
                         kind="ExternalInput").ap()
    Wkn = nc.dram_tensor("Wkn", [128, KKV, 2 * NOPE], BF16,
                         kind="ExternalInput").ap()
    Wv = nc.dram_tensor("Wv", [128, KKV, 2 * VDIM], BF16,
                        kind="ExternalInput").ap()
    Wo = nc.dram_tensor("Wo", [128, 2, D_MODEL], BF16, kind="ExternalInput").ap()
    cosT2 = nc.dram_tensor("cosT2", [128, SEQ], BF16, kind="ExternalInput").ap()
    sinT2 = nc.dram_tensor("sinT2", [128, SEQ], BF16, kind="ExternalInput").ap()
    permT2 = nc.dram_tensor("permT2", [128, 128], BF16, kind="ExternalInput").ap()
    ones = nc.dram_tensor("ones", [128, 1], BF16, kind="ExternalInput").ap()
    rq = nc.dram_tensor("rq", [1, SEQ], F32, kind="ExternalInput").ap()

    out = nc.dram_tensor("out", [SEQ, D_MODEL], BF16, kind="ExternalOutput").ap()

    with tile.TileContext(nc) as tc, ExitStack() as ctx:
        const = ctx.enter_context(tc.tile_pool(name="const", bufs=1))
        big = ctx.enter_context(tc.tile_pool(name="big", bufs=1))
        tmp1 = ctx.enter_context(tc.tile_pool(name="tmp1", bufs=1))
        work = ctx.enter_context(tc.tile_pool(name="work", bufs=2))
        exp_pool = ctx.enter_context(tc.tile_pool(name="expp", bufs=3))
        opool = ctx.enter_context(tc.tile_pool(name="opool", bufs=2))
        psum = ctx.enter_context(tc.tile_pool(name="psum", bufs=1, space="PSUM"))

        # Tiny DMAs on the ACT HWDGE queue (slip into DMA_ENGINES gaps);
        # big DMAs in consumption order on the SP HWDGE queue.
        sb_ones = const.tile([128, 1], BF16, tag="ones")
        nc.scalar.dma_start(sb_ones[:], ones)
        sb_perm2 = const.tile([128, 128], BF16, tag="perm2")
        nc.scalar.dma_start(sb_perm2[:], permT2)
        sb_rq = const.tile([1, SEQ], F32, tag="rq")
        nc.scalar.dma_start(sb_rq[:], rq)

        sb_wqb = big.tile([128, KQ, 2 * QHD], BF16, tag="wqb")
        nc.sync.dma_start(sb_wqb[:], Wqb)
        # tnT streams through one half-sized buffer pair (SBUF pressure):
        # half A = tokens 0:1024, half B = 1024:2048 (B loaded late, its
        # consumers are r00 fillers / r1x passes)
        tnT_a = big.tile([128, KQ, 1024], BF16, tag="tnT", name="tnT_a")
        nc.sync.dma_start(tnT_a[:, :, 0:512], tnT[:, :, 0:512])
        sb_compT = big.tile([128, KKV, SEQ], BF16, tag="compT")
        nc.sync.dma_start(sb_compT[:], compT)
        nc.sync.dma_start(tnT_a[:, :, 512:1024], tnT[:, :, 512:1024])
        sb_cos2 = const.tile([128, SEQ], BF16, tag="cos2")
        nc.sync.dma_start(sb_cos2[:], cosT2)
        sb_sin2 = const.tile([128, SEQ], BF16, tag="sin2")
        nc.sync.dma_start(sb_sin2[:], sinT2)
        sb_kpe = big.tile([ROPE, SEQ], BF16, tag="kpe")
        nc.sync.dma_start(sb_kpe[:], kpeT)
        sb_wkn = big.tile([128, KKV, 2 * NOPE], BF16, tag="wkn")
        nc.sync.dma_start(sb_wkn[:], Wkn)
        sb_wv = big.tile([128, KKV, 2 * VDIM], BF16, tag="wv")
        nc.sync.dma_start(sb_wv[:], Wv)
        tnT_b = big.tile([128, KQ, 1024], BF16, tag="tnT2", name="tnT_b")
        nc.sync.dma_start(tnT_b[:], tnT[:, :, 1024:SEQ])
        sb_wo = big.tile([128, 2, D_MODEL], BF16, tag="wo")
        nc.sync.dma_start(sb_wo[:], Wo)

        # q norm scale broadcast to all partitions (gpsimd)
        rq_b = tmp1.tile([128, SEQ], F32, tag="rqb")
        nc.gpsimd.partition_broadcast(rq_b[:], sb_rq[:])

        # ---- k_nope^T per head (comp already rms-normalized in L1)
        k_nope = [big.tile([128, SEQ], BF16, tag=f"kn{h}", name=f"kn{h}")
                  for h in range(2)]

        def kn_proj(h, n):
            nsl = slice(n * 512, (n + 1) * 512)
            acc = psum.tile([128, 512], F32, tag="scores", bufs=3, name="kacc")
            for k in range(KKV):
                nc.tensor.matmul(acc[:], sb_wkn[:, k, h * 128:(h + 1) * 128],
                                 sb_compT[:, k, nsl],
                                 start=(k == 0), stop=(k == KKV - 1))
            if (h * 4 + n) % 2 == 0:
                nc.scalar.copy(k_nope[h][:, nsl], acc[:])
            else:
                nc.vector.tensor_copy(k_nope[h][:, nsl], acc[:])

        # ---- v natural [tok, 2*vd]; ACT drains it while q keeps DVE busy
        v_nat = big.tile([128, ST, 2 * VDIM], BF16, tag="vnat")

        def v_proj(t):
            acc = psum.tile([128, 2 * VDIM], F32, tag="scores", bufs=3,
                            name="vacc")
            for k in range(KKV):
                nc.tensor.matmul(acc[:], sb_compT[:, k, t * 128:(t + 1) * 128],
                                 sb_wv[:, k, :],
                                 start=(k == 0), stop=(k == KKV - 1))
            nc.scalar.copy(v_nat[:, t, :], acc[:])

        # ---- q^T (rq folded into the PSUM-drain mul)
        q_nope = [big.tile([128, SEQ], BF16, tag=f"qn{h}", name=f"qn{h}")
                  for h in range(2)]
        qpe_raw = tmp1.tile([128, SEQ], BF16, tag="qpe_raw")
        qswap = tmp1.tile([128, SEQ], BF16, tag="qswap")
        qpe2 = qpe_raw  # rope overwrites the raw rows in place
        qpe_h1 = big.tile([ROPE, SEQ], BF16, tag="qpeh1")

        def q_proj(n, m):
            nsl = slice(n * 512, (n + 1) * 512)
            src_t = tnT_a if n < 2 else tnT_b
            lsl = slice((n % 2) * 512, (n % 2 + 1) * 512)
            acc = psum.tile([128, 512], F32, tag="scores", bufs=3, name="qacc")
            for k in range(KQ):
                nc.tensor.matmul(acc[:], sb_wqb[:, k, m * 128:(m + 1) * 128],
                                 src_t[:, k, lsl],
                                 start=(k == 0), stop=(k == KQ - 1))
            dst = q_nope[m] if m < 2 else qpe_raw
            nc.vector.tensor_mul(dst[:, nsl], acc[:], rq_b[:, nsl])

        def rope_n(n):
            sl = slice(n * 512, (n + 1) * 512)
            sw = psum.tile([128, 512], F32, tag="scores", bufs=3, name="sw")
            nc.tensor.matmul(sw[:], sb_perm2[:], qpe_raw[:, sl],
                             start=True, stop=True)
            nc.vector.tensor_copy(qswap[:, sl], sw[:])
            nc.vector.tensor_mul(qpe2[:, sl], qpe_raw[:, sl], sb_cos2[:, sl])
            nc.vector.tensor_mul(qswap[:, sl], qswap[:, sl], sb_sin2[:, sl])
            nc.vector.tensor_add(qpe2[:, sl], qpe2[:, sl], qswap[:, sl])
            # (qpe2 aliases qpe_raw; the perm matmul consumed the raw values)
            # h1 rope rows to a base-0 tile (partition shift needs DMA)
            nc.sync.dma_start(qpe_h1[:, sl], qpe2[ROPE:128, sl])

        def qpe_of(h):
            return qpe2[0:ROPE, :] if h == 0 else qpe_h1[:, :]

        # lead-in emission: only what r00 (= sb0, h0) needs up front:
        # q m0/m2 + rope for tokens 0:1024, k_nope[0], v. The h1 tiles
        # (q m1, k_nope[1]) and the sb1 q-tiles ride r00/r01 as fillers.
        q_proj(0, 0)
        q_proj(0, 2)
        for n in range(4):
            kn_proj(0, n)
        q_proj(1, 0)
        q_proj(1, 2)
        rope_n(0)
        rope_n(1)
        for t in range(ST):
            v_proj(t)

        f00 = [lambda: q_proj(0, 1), lambda: q_proj(1, 1)]
        f00 += [lambda n=n: kn_proj(1, n) for n in range(4)]
        f00 += [lambda: None, lambda: None]

        f01x = [lambda n=n, m=m: q_proj(n, m) for n in (2, 3) for m in (0, 2, 1)]
        f01x.insert(3, lambda: rope_n(2))
        f01x.append(lambda: rope_n(3))

        # ---- attention: scores^T -> exp pairs -> av (PSUM) + den (DVE tree)
        # av accumulates one 512-query psl at a time (1 bank) so scores get
        # 3 buffers; psl1 replays the kept exp tiles as phase B, interleaved
        # into the next pass. den: DVE pairwise adds + 2 tiny ones-matmuls.
        att_sb = [big.tile([128, 2, SB], BF16, tag=f"att{b}", name=f"att{b}")
                  for b in range(NSB)]

        def av_psl(h, tp, e2, psl_i, av_t):
            psl = slice(psl_i * 512, (psl_i + 1) * 512)
            for half in range(2):
                nc.tensor.matmul(av_t[:],
                                 v_nat[:, 2 * tp + half,
                                       h * VDIM:(h + 1) * VDIM],
                                 e2[:, half, psl],
                                 start=(tp == 0 and half == 0),
                                 stop=(tp == ST // 2 - 1 and half == 1))

        def den_psl(den_acc, psl_i):
            """ones-matmul partition sums of the accumulated exp pairs."""
            psl = slice(psl_i * 512, (psl_i + 1) * 512)
            den_ps = psum.tile([1, 512], F32, tag="den", bufs=1, name="den_ps")
            for half in range(2):
                nc.tensor.matmul(den_ps[:], sb_ones[:], den_acc[:, half, psl],
                                 start=(half == 0), stop=(half == 1))
            return den_ps

        def finish_psl(sb_i, h, psl_i, av_t, den_ps):
            psl = slice(psl_i * 512, (psl_i + 1) * 512)
            den_r = work.tile([1, 512], F32, tag="denr", name="den_r")
            nc.vector.reciprocal(den_r[:], den_ps[:])
            den_b = work.tile([128, 512], F32, tag="denb", bufs=1, name="den_b")
            nc.gpsimd.partition_broadcast(den_b[:], den_r[:])
            nc.vector.tensor_mul(att_sb[sb_i][:, h, psl], av_t[:], den_b[:])

        def attention_pass(sb_i, h, fillers=()):
            fillers = list(fillers)
            s0 = sb_i * SB
            state = {}

            def lazy_av():
                if "av0" not in state:
                    state["av0"] = psum.tile([128, 512], F32, tag="av",
                                             bufs=1, name="av0")
                return state["av0"]

            den_acc = exp_pool.tile([128, 2, SB], BF16, tag="dacc", bufs=1,
                                    name="den_acc")
            e2s = []
            pending = []
            for tp in range(ST // 2):
                # deferred work first so it isn't stuck behind blocked sc allocs
                if len(pending) >= 2:
                    p = pending.pop(0)
                    av_psl(h, p, e2s[p], 0, lazy_av())
                    if p == 0:
                        nc.vector.tensor_add(den_acc[:], e2s[0][:], e2s[1][:])
                    elif p >= 2:
                        nc.vector.tensor_add(den_acc[:], den_acc[:],
                                             e2s[p][:])
                if fillers:
                    fillers.pop(0)()
                e2 = exp_pool.tile([128, 2, SB], BF16, tag="exp2", bufs=8,
                                   name="e2")
                e2s.append(e2)
                for half in range(2):
                    t = 2 * tp + half
                    sc = psum.tile([128, SB], F32, tag="scores", bufs=3,
                                   name="sc")
                    for n2 in range(SB // 512):
                        sl = slice(s0 + n2 * 512, s0 + (n2 + 1) * 512)
                        psl = slice(n2 * 512, (n2 + 1) * 512)
                        nc.tensor.matmul(sc[:, psl],
                                         k_nope[h][:, t * 128:(t + 1) * 128],
                                         q_nope[h][:, sl],
                                         start=True, stop=False)
                        nc.tensor.matmul(sc[:, psl],
                                         sb_kpe[:, t * 128:(t + 1) * 128],
                                         qpe_of(h)[:, sl],
                                         start=False, stop=True)
                    nc.scalar.activation(e2[:, half, :], sc[:], AF.Exp,
                                         scale=SCALE)
                pending.append(tp)
            av0 = lazy_av()
            for p in pending:
                av_psl(h, p, e2s[p], 0, av0)
                if p >= 2:
                    nc.vector.tensor_add(den_acc[:], den_acc[:], e2s[p][:])
            for f in fillers:
                f()
            finish_psl(sb_i, h, 0, av0, den_psl(den_acc, 0))
            return den_acc, e2s

        def phase_b(sb_i, h, state):
            den_acc, e2s = state
            av1 = psum.tile([128, 512], F32, tag="av", bufs=1, name="av1")
            for tp in range(ST // 2):
                av_psl(h, tp, e2s[tp], 1, av1)
            finish_psl(sb_i, h, 1, av1, den_psl(den_acc, 1))

        def oproj_ms(sb_i, ms, engines):
            s0 = sb_i * SB
            att2 = att_sb[sb_i]
            o = opool.tile([128, D_MODEL], BF16, tag="osb", bufs=2, name="o")
            for n in range(D_MODEL // 512):
                acc = psum.tile([128, 512], F32, tag="scores", bufs=3,
                                name="oacc")
                for h in range(2):
                    nc.tensor.matmul(acc[:],
                                     att2[:, h, ms * 128:(ms + 1) * 128],
                                     sb_wo[:, h, n * 512:(n + 1) * 512],
                                     start=(h == 0), stop=(h == 1))
                eng = engines[n % len(engines)]
                if eng is nc.scalar:
                    eng.copy(o[:, n * 512:(n + 1) * 512], acc[:])
                else:
                    eng.tensor_copy(o[:, n * 512:(n + 1) * 512], acc[:])
            nc.sync.dma_start(out[s0 + ms * 128: s0 + (ms + 1) * 128, :],
                              o[:])

        # fillers: prev pass's psl1 replay (phase B) leads each later pass;
        # oproj(0) rides r10/r11
        r00 = attention_pass(0, 0, f00)
        f01 = [lambda: phase_b(0, 0, r00)] + f01x
        r01 = attention_pass(0, 1, f01)

        f10 = [lambda: phase_b(0, 1, r01)]
        f10 += [lambda ms=ms: oproj_ms(0, ms, (nc.vector, nc.scalar))
                for ms in range(4)]
        r10 = attention_pass(1, 0, f10)

        f11 = [lambda: phase_b(1, 0, r10)]
        f11 += [lambda ms=ms: oproj_ms(0, ms, (nc.vector, nc.scalar))
                for ms in range(4, 8)]
        r11 = attention_pass(1, 1, f11)

        # tail: oproj(1) ms0-3 only needs the psl0 halves (cols 0:512), which
        # attention_pass already finished; ACT is free again here
        for ms in range(4):
            oproj_ms(1, ms, (nc.vector, nc.scalar))
        phase_b(1, 1, r11)
        for ms in range(4, 8):
            oproj_ms(1, ms, (nc.vector, nc.scalar))

    nc.compile()
    return nc


# --------------------------------------------------------------------------
# Host orchestration
# --------------------------------------------------------------------------

def _prep(x, freqs_cis, Wqa, qln, Wqb, Wkva, kvln, Wkvb, Wo):
    """Host-side sharding prep (numpy reshapes/casts only)."""
    KD = D_MODEL // 128
    xT = np.ascontiguousarray(x[0].T)                         # [D, S] f32
    xTp = _pack(xT.astype(BF), KD)                            # [128, KD, S]
    cos = freqs_cis[..., 0].astype(np.float32)                # [S, 32]
    sin = freqs_cis[..., 1].astype(np.float32)
    cosT = np.repeat(np.ascontiguousarray(cos.T), 2, axis=0)  # [64, S]
    sinT = np.repeat(np.ascontiguousarray(sin.T), 2, axis=0)

    Wqa_p = _pack(Wqa.astype(BF), KD)
    Wkva_p = _pack(Wkva.astype(BF), KD)

    Wqb_f = (Wqb * qln[:, None]).astype(np.float32)
    Wkvb_f = (Wkvb * kvln[:, None]).astype(np.float32)
    Wqb_hd = Wqb_f.reshape(Q_LORA, NH, QHD)
    Wkvb_hd = Wkvb_f.reshape(KV_LORA, NH, NOPE + VDIM)
    Wo_hd = Wo.reshape(NH, VDIM, D_MODEL)
    l2_per_core = []
    for c in range(N_CORES):
        hs = [2 * c, 2 * c + 1]
        wqb_c = np.concatenate(
            [Wqb_hd[:, hs[0], :NOPE], Wqb_hd[:, hs[1], :NOPE],
             Wqb_hd[:, hs[0], NOPE:], Wqb_hd[:, hs[1], NOPE:]], axis=1)
        wkn_c = np.concatenate([Wkvb_hd[:, h, :NOPE] for h in hs], axis=1)
        wv_c = np.concatenate([Wkvb_hd[:, h, NOPE:] for h in hs], axis=1)
        wo_c = np.concatenate([Wo_hd[h] for h in hs], axis=0)
        l2_per_core.append(dict(
            Wqb=_pack(wqb_c.astype(BF), Q_LORA // 128),
            Wkn=_pack(wkn_c.astype(BF), KV_LORA // 128),
            Wv=_pack(wv_c.astype(BF), KV_LORA // 128),
            Wo=_pack(wo_c.astype(BF), 2),
        ))

    return dict(xTp=xTp, cosT=cosT, sinT=sinT,
                Wqa=Wqa_p, Wkva=Wkva_p,
                ones=np.ones((128, 1), BF),
                perm64=_perm_rope_T(ROPE), perm128=_perm_rope_T(128),
                cosT2=np.concatenate([cosT, cosT], axis=0).astype(BF),
                sinT2=np.concatenate([sinT, sinT], axis=0).astype(BF),
                l2=l2_per_core)


def _get_programs():
    if "l1" not in _CACHE:
        _CACHE["l1"] = build_l1()
    if "l2" not in _CACHE:
        _CACHE["l2"] = build_l2()
    return _CACHE["l1"], _CACHE["l2"]


def kernel(x, mask, freqs_cis, Wqa, qln, Wqb, Wkva, kvln, Wkvb, Wo,
           _trace=False, _tmpdirs=None):
    p = _prep(x, freqs_cis, Wqa, qln, Wqb, Wkva, kvln, Wkvb, Wo)
    l1, l2 = _get_programs()

    in1 = []
    for c in range(N_CORES):
        sl = slice(c * S_LOC, (c + 1) * S_LOC)
        in1.append(dict(
            xT=np.ascontiguousarray(p["xTp"][:, :, sl]),
            Wqa=p["Wqa"], Wkva=p["Wkva"],
            cosT=np.ascontiguousarray(p["cosT"][:, sl]),
            sinT=np.ascontiguousarray(p["sinT"][:, sl]),
            permT=p["perm64"], ones=p["ones"],
        ))
    kw1 = {}
    if _trace:
        kw1 = dict(trace=True, tmpdir=(_tmpdirs or [None, None])[0])
    r1 = run_bass_kernel_spmd(l1, in1, core_ids=list(range(N_CORES)), **kw1)

    tnT = np.concatenate([r1.results[c]["tnT"] for c in range(N_CORES)], axis=2)
    compT = np.concatenate([r1.results[c]["compT"] for c in range(N_CORES)],
                           axis=2)
    kpeT = np.concatenate([r1.results[c]["kpeT"] for c in range(N_CORES)],
                          axis=1)
    sqq = np.concatenate([r1.results[c]["sqq"] for c in range(N_CORES)], axis=1)
    rq_v = 1.0 / np.sqrt(sqq[0].astype(np.float64) / Q_LORA + EPS)
    rq_row = rq_v.astype(np.float32)[None, :]                   # [1, SEQ]

    in2 = []
    for c in range(N_CORES):
        d = dict(tnT=tnT, compT=compT, kpeT=kpeT,
                 cosT2=p["cosT2"], sinT2=p["sinT2"], permT2=p["perm128"],
                 ones=p["ones"], rq=rq_row)
        d.update(p["l2"][c])
        in2.append(d)
    kw2 = {}
    if _trace:
        kw2 = dict(trace=True, tmpdir=(_tmpdirs or [None, None])[1])
    r2 = run_bass_kernel_spmd(l2, in2, core_ids=list(range(N_CORES)), **kw2)

    acc = np.zeros((SEQ, D_MODEL), np.float64)
    for c in range(N_CORES):
        acc += r2.results[c]["out"].astype(np.float64)
    out = acc.astype(np.float32)[None]  # [1, S, D]

    kernel._last = (r1, r2)
    return out


# revision 66
# speedup vs baseline: 1.0087x; 1.0087x over previous
"""MLA attention (DeepSeek-style, LoRA Q/KV) on 8 Trainium2 NeuronCores.

All matmuls in bf16 (fp8 fails the 2e-2 absmax gate: quantization noise of
a random matmul does not average down relative to its output).

Two SPMD launches (host gathers between them):
  L1 (sequence-parallel, 256 tokens/core): LoRA-A projections. comp is
  rms-normalized on-chip (its scale is ready early); the q-path ships RAW
  with per-token sq-sums, and the host computes rq = rsqrt(mean+eps) for
  free between launches (kills the L1 tail).
  L2 (tensor-parallel, 2 heads/core): q/k/v LoRA-B projections (rq folded
  into the PSUM-drain copies), scores^T per key tile, exp(bf16) pairs,
  attn@v in PSUM, softmax denominator via a DVE add-tree over the exp
  pairs plus tiny ones-matmuls (frees PSUM banks for triple-buffered
  scores), per-head normalize, output projection. Host sums 8 partials.

Schedule: scores get 3 PSUM buffers (av/den accumulate one 512-query psl
at a time, 1 bank each; psl1 replays the kept exp tiles as a phase B
interleaved into the next pass). Late q-tiles, v-tiles and the first
output projection ride the attention tp-loops as fillers so the in-order
PE stream never drains. All DMAs use partition-major host-packed layouts
(long contiguous descriptors), big ones on the SP queue in consumption
order, tiny ones on the ACT queue.
"""

import math
from contextlib import ExitStack

import numpy as np
import ml_dtypes

import concourse.bass as bass
import concourse.mybir as mybir
import concourse.tile as tile
from concourse import bacc
from concourse.bass_utils import run_bass_kernel_spmd

BF = ml_dtypes.bfloat16
F32 = mybir.dt.float32
BF16 = mybir.dt.bfloat16
AF = mybir.ActivationFunctionType
ALU = mybir.AluOpType

D_MODEL = 2048
NH = 16
Q_LORA = 1536
KV_LORA = 512
ROPE = 64
NOPE = 128
VDIM = 128
QHD = NOPE + ROPE  # 192
SEQ = 2048
N_CORES = 8
S_LOC = SEQ // N_CORES  # 256 tokens per core in L1
EPS = 1e-6
SCALE = 1.0 / math.sqrt(128.0)  # 1/sqrt(HEAD_DIM), as in the reference

_CACHE = {}


def _perm_rope_T(n):
    """lhsT for P @ v where (P@v)[2i] = -v[2i+1], (P@v)[2i+1] = v[2i]."""
    P = np.zeros((n, n), np.float32)
    for i in range(n // 2):
        P[2 * i, 2 * i + 1] = -1.0
        P[2 * i + 1, 2 * i] = 1.0
    return np.ascontiguousarray(P.T).astype(BF)


def _pack(w, k):
    """[k*128, n] -> partition-major [128, k, n] (one long row per partition)."""
    n = w.shape[-1]
    return np.ascontiguousarray(w.reshape(k, 128, n).transpose(1, 0, 2))


# --------------------------------------------------------------------------
# Launch 1: sequence-sharded LoRA-A projections (+ comp norm, q sq-sums)
# --------------------------------------------------------------------------

def build_l1():
    nc = bacc.Bacc("TRN2", target_bir_lowering=False, debug=False,
                   enable_asserts=True, num_devices=N_CORES)
    KD = D_MODEL // 128   # 16
    MQ = Q_LORA // 128    # 12

    xT = nc.dram_tensor("xT", [128, KD, S_LOC], BF16, kind="ExternalInput").ap()
    Wqa = nc.dram_tensor("Wqa", [128, KD, Q_LORA], BF16, kind="ExternalInput").ap()
    Wkva = nc.dram_tensor("Wkva", [128, KD, 576], BF16, kind="ExternalInput").ap()
    cosT = nc.dram_tensor("cosT", [ROPE, S_LOC], F32, kind="ExternalInput").ap()
    sinT = nc.dram_tensor("sinT", [ROPE, S_LOC], F32, kind="ExternalInput").ap()
    permT = nc.dram_tensor("permT", [ROPE, ROPE], BF16, kind="ExternalInput").ap()
    ones = nc.dram_tensor("ones", [128, 1], BF16, kind="ExternalInput").ap()

    tnT = nc.dram_tensor("tnT", [128, MQ, S_LOC], BF16, kind="ExternalOutput").ap()
    compT = nc.dram_tensor("compT", [128, 4, S_LOC], BF16,
                           kind="ExternalOutput").ap()
    kpeT = nc.dram_tensor("kpeT", [ROPE, S_LOC], BF16, kind="ExternalOutput").ap()
    sqq = nc.dram_tensor("sqq", [1, S_LOC], F32, kind="ExternalOutput").ap()

    with tile.TileContext(nc) as tc, ExitStack() as ctx:
        const = ctx.enter_context(tc.tile_pool(name="const", bufs=1))
        big = ctx.enter_context(tc.tile_pool(name="big", bufs=1))
        work = ctx.enter_context(tc.tile_pool(name="work", bufs=3))
        ps = ctx.enter_context(tc.tile_pool(name="ps", bufs=3, space="PSUM"))
        ps1 = ctx.enter_context(tc.tile_pool(name="ps1", bufs=1, space="PSUM"))

        # big DMAs first on SP queue; tiny consts ride the ACT queue
        sb_xT = big.tile([128, KD, S_LOC], BF16, tag="xT")
        sb_wkva = big.tile([128, KD, 576], BF16, tag="wkva")
        for k0, k1 in ((0, 4), (4, 8), (8, KD)):
            nc.sync.dma_start(sb_xT[:, k0:k1, :], xT[:, k0:k1, :])
            nc.sync.dma_start(sb_wkva[:, k0:k1, :], Wkva[:, k0:k1, :])
        sb_wqa = big.tile([128, KD, Q_LORA], BF16, tag="wqa")
        for nchunk in range(3):  # 512-col chunks so q m-tiles start early
            nsl = slice(nchunk * 512, (nchunk + 1) * 512)
            nc.sync.dma_start(sb_wqa[:, :, nsl], Wqa[:, :, nsl])
        sb_cos = const.tile([ROPE, S_LOC], F32, tag="cos")
        nc.scalar.dma_start(sb_cos[:], cosT)
        sb_sin = const.tile([ROPE, S_LOC], F32, tag="sin")
        nc.scalar.dma_start(sb_sin[:], sinT)
        sb_perm = const.tile([ROPE, ROPE], BF16, tag="perm")
        nc.scalar.dma_start(sb_perm[:], permT)
        sb_ones = const.tile([128, 1], BF16, tag="ones")
        nc.scalar.dma_start(sb_ones[:], ones)

        # ---- ckv.T: 4 comp m-tiles + k_pe [64]
        # kv norm applied HERE (its scale is ready early, no tail impact)
        c_raw = big.tile([128, 4, S_LOC], BF16, tag="craw")
        c_sq = big.tile([128, 4, S_LOC], BF16, tag="csq")
        for m in range(4):
            acc = ps.tile([128, S_LOC], F32, tag="acc")
            for k in range(KD):
                nc.tensor.matmul(acc[:], sb_wkva[:, k, m * 128:(m + 1) * 128],
                                 sb_xT[:, k, :],
                                 start=(k == 0), stop=(k == KD - 1))
            nc.scalar.copy(c_raw[:, m, :], acc[:])
            nc.vector.tensor_mul(c_sq[:, m, :], c_raw[:, m, :], c_raw[:, m, :])

        # k_pe rows 512:576 -> [64, S]; rope it (k_pe is not normalized)
        kpe_acc = ps1.tile([64, S_LOC], F32, tag="kpe")
        for k in range(KD):
            nc.tensor.matmul(kpe_acc[:], sb_wkva[:, k, 512:576],
                             sb_xT[:, k, :],
                             start=(k == 0), stop=(k == KD - 1))
        kpe_sb = work.tile([64, S_LOC], BF16, tag="kpesb")
        nc.scalar.copy(kpe_sb[:], kpe_acc[:])
        swap_ps = ps1.tile([64, S_LOC], F32, tag="swap")
        nc.tensor.matmul(swap_ps[:], sb_perm[:], kpe_sb[:], start=True, stop=True)
        kc = work.tile([64, S_LOC], F32, tag="kc")
        nc.vector.tensor_mul(kc[:], kpe_sb[:], sb_cos[:])
        ks = work.tile([64, S_LOC], F32, tag="ks")
        nc.vector.tensor_mul(ks[:], swap_ps[:], sb_sin[:])
        kout = work.tile([64, S_LOC], BF16, tag="kout")
        nc.vector.tensor_add(kout[:], kc[:], ks[:])
        nc.sync.dma_start(kpeT, kout[:])

        # kv rms scale (partition sum via ones-matmul) and apply
        sqkv_ps = ps1.tile([1, S_LOC], F32, tag="sqkv")
        for m in range(4):
            nc.tensor.matmul(sqkv_ps[:], sb_ones[:], c_sq[:, m, :],
                             start=(m == 0), stop=(m == 3))
        eps_t = const.tile([1, 1], F32, tag="eps")
        nc.vector.memset(eps_t[:], EPS)
        sroot = work.tile([1, S_LOC], F32, tag="sroot")
        nc.scalar.activation(sroot[:], sqkv_ps[:], AF.Sqrt,
                             bias=eps_t[:], scale=1.0 / KV_LORA)
        rec = work.tile([1, S_LOC], F32, tag="rec")
        nc.vector.reciprocal(rec[:], sroot[:])
        rkv_b = work.tile([128, S_LOC], F32, tag="rkvb")
        nc.gpsimd.partition_broadcast(rkv_b[:], rec[:])
        o_cn = big.tile([128, 4, S_LOC], BF16, tag="ocn")
        for m in range(4):
            nc.vector.tensor_mul(o_cn[:, m, :], c_raw[:, m, :], rkv_b[:])
        nc.sync.dma_start(compT, o_cn[:])

        # ---- t.T: 12 m-tiles of [128, 256]; RAW bf16 out + sq-sums
        # (sq partition-sums accumulate per-m so the tail is one matmul)
        o_tn = big.tile([128, MQ, S_LOC], BF16, tag="otn")
        t_sq = big.tile([128, MQ, S_LOC], BF16, tag="tsq")
        sqq_ps = ps1.tile([1, S_LOC], F32, tag="sqq")
        for m in range(MQ):
            acc = ps.tile([128, S_LOC], F32, tag="acc")
            for k in range(KD):
                nc.tensor.matmul(acc[:], sb_wqa[:, k, m * 128:(m + 1) * 128],
                                 sb_xT[:, k, :],
                                 start=(k == 0), stop=(k == KD - 1))
            nc.scalar.copy(o_tn[:, m, :], acc[:])
            nc.vector.tensor_mul(t_sq[:, m, :], o_tn[:, m, :], o_tn[:, m, :])
            nc.tensor.matmul(sqq_ps[:], sb_ones[:], t_sq[:, m, :],
                             start=(m == 0), stop=(m == MQ - 1))
            if m % 2 == 1:  # write in 2-m-tile chunks to overlap DMA
                nc.sync.dma_start(tnT[:, m - 1:m + 1, :],
                                  o_tn[:, m - 1:m + 1, :])

        sqq_sb = work.tile([1, S_LOC], F32, tag="sqqsb")
        nc.scalar.copy(sqq_sb[:], sqq_ps[:])
        nc.sync.dma_start(sqq, sqq_sb[:])

    nc.compile()
    return nc


# --------------------------------------------------------------------------
# Launch 2: head-sharded attention (2 heads per core)
# --------------------------------------------------------------------------

def build_l2():
    nc = bacc.Bacc("TRN2", target_bir_lowering=False, debug=False,
                   enable_asserts=True, num_devices=N_CORES)
    KQ = Q_LORA // 128    # 12
    KKV = KV_LORA // 128  # 4
    ST = SEQ // 128       # 16 key tiles
    SB = 1024             # query block
    NSB = SEQ // SB       # 2

    tnT = nc.dram_tensor("tnT", [128, KQ, SEQ], BF16, kind="ExternalInput").ap()
    compT = nc.dram_tensor("compT", [128, KKV, SEQ], BF16,
                           kind="ExternalInput").ap()
    kpeT = nc.dram_tensor("kpeT", [ROPE, SEQ], BF16, kind="ExternalInput").ap()
    # Wqb cols reordered [h0 nope | h1 nope | h0 rope | h1 rope], qln folded
    Wqb = nc.dram_tensor("Wqb", [128, KQ, 2 * QHD], BF16,
                         kind="ExternalInput").ap()
    Wkn = nc.dram_tensor("Wkn", [128, KKV, 2 * NOPE], BF16,
                         kind="ExternalInput").ap()
    Wv = nc.dram_tensor("Wv", [128, KKV, 2 * VDIM], BF16,
                        kind="ExternalInput").ap()
    Wo = nc.dram_tensor("Wo", [128, 2, D_MODEL], BF16, kind="ExternalInput").ap()
    cosT2 = nc.dram_tensor("cosT2", [128, SEQ], BF16, kind="ExternalInput").ap()
    sinT2 = nc.dram_tensor("sinT2", [128, SEQ], BF16, kind="ExternalInput").ap()
    permT2 = nc.dram_tensor("permT2", [128, 128], BF16, kind="ExternalInput").ap()
    ones = nc.dram_tensor("ones", [128, 1], BF16, kind="ExternalInput").ap()
    rq = nc.dram_tensor("rq", [1, SEQ], F32, kind="ExternalInput").ap()

    out = nc.dram_tensor("out", [SEQ, D_MODEL], BF16, kind="ExternalOutput").ap()

    with tile.TileContext(nc) as tc, ExitStack() as ctx:
        const = ctx.enter_context(tc.tile_pool(name="const", bufs=1))
        big = ctx.enter_context(tc.tile_pool(name="big", bufs=1))
        tmp1 = ctx.enter_context(tc.tile_pool(name="tmp1", bufs=1))
        work = ctx.enter_context(tc.tile_pool(name="work", bufs=2))
        exp_pool = ctx.enter_context(tc.tile_pool(name="expp", bufs=3))
        opool = ctx.enter_context(tc.tile_pool(name="opool", bufs=2))
        psum = ctx.enter_context(tc.tile_pool(name="psum", bufs=1, space="PSUM"))

        # Tiny DMAs on the ACT HWDGE queue (slip into DMA_ENGINES gaps);
        # big DMAs in consumption order on the SP HWDGE queue.
        sb_ones = const.tile([128, 1], BF16, tag="ones")
        nc.scalar.dma_start(sb_ones[:], ones)
        sb_perm2 = const.tile([128, 128], BF16, tag="perm2")
        nc.scalar.dma_start(sb_perm2[:], permT2)
        sb_rq = const.tile([1, SEQ], F32, tag="rq")
        nc.scalar.dma_start(sb_rq[:], rq)

        sb_wqb = big.tile([128, KQ, 2 * QHD], BF16, tag="wqb")
        nc.sync.dma_start(sb_wqb[:], Wqb)
        # tnT streams through one half-sized buffer pair (SBUF pressure):
        # half A = tokens 0:1024, half B = 1024:2048 (B loaded late, its
        # consumers are r00 fillers / r1x passes)
        tnT_a = big.tile([128, KQ, 1024], BF16, tag="tnT", name="tnT_a")
        nc.sync.dma_start(tnT_a[:, :, 0:512], tnT[:, :, 0:512])
        sb_compT = big.tile([128, KKV, SEQ], BF16, tag="compT")
        nc.sync.dma_start(sb_compT[:], compT)
        nc.sync.dma_start(tnT_a[:, :, 512:1024], tnT[:, :, 512:1024])
        sb_cos2 = const.tile([128, SEQ], BF16, tag="cos2")
        nc.sync.dma_start(sb_cos2[:], cosT2)
        sb_sin2 = const.tile([128, SEQ], BF16, tag="sin2")
        nc.sync.dma_start(sb_sin2[:], sinT2)
        sb_kpe = big.tile([ROPE, SEQ], BF16, tag="kpe")
        nc.sync.dma_start(sb_kpe[:], kpeT)
        sb_wkn = big.tile([128, KKV, 2 * NOPE], BF16, tag="wkn")
        nc.sync.dma_start(sb_wkn[:], Wkn)
        sb_wv = big.tile([128, KKV, 2 * VDIM], BF16, tag="wv")
        nc.sync.dma_start(sb_wv[:], Wv)
        tnT_b = big.tile([128, KQ, 1024], BF16, tag="tnT2", name="tnT_b")
        nc.sync.dma_start(tnT_b[:], tnT[:, :, 1024:SEQ])
        sb_wo = big.tile([128, 2, D_MODEL], BF16, tag="wo")
        nc.sync.dma_start(sb_wo[:], Wo)

        # q norm scale broadcast to all partitions (gpsimd)
        rq_b = tmp1.tile([128, SEQ], F32, tag="rqb")
        nc.gpsimd.partition_broadcast(rq_b[:], sb_rq[:])

        # ---- k_nope^T per head (comp already rms-normalized in L1)
        k_nope = [big.tile([128, SEQ], BF16, tag=f"kn{h}", name=f"kn{h}")
                  for h in range(2)]

        def kn_proj(h, n):
            nsl = slice(n * 512, (n + 1) * 512)
            acc = psum.tile([128, 512], F32, tag="scores", bufs=3, name="kacc")
            for k in range(KKV):
                nc.tensor.matmul(acc[:], sb_wkn[:, k, h * 128:(h + 1) * 128],
                                 sb_compT[:, k, nsl],
                                 start=(k == 0), stop=(k == KKV - 1))
            if (h * 4 + n) % 2 == 0:
                nc.scalar.copy(k_nope[h][:, nsl], acc[:])
            else:
                nc.vector.tensor_copy(k_nope[h][:, nsl], acc[:])

        # ---- v natural [tok, 2*vd]; ACT drains it while q keeps DVE busy
        v_nat = big.tile([128, ST, 2 * VDIM], BF16, tag="vnat")

        def v_proj(t):
            acc = psum.tile([128, 2 * VDIM], F32, tag="scores", bufs=3,
                            name="vacc")
            for k in range(KKV):
                nc.tensor.matmul(acc[:], sb_compT[:, k, t * 128:(t + 1) * 128],
                                 sb_wv[:, k, :],
                                 start=(k == 0), stop=(k == KKV - 1))
            nc.scalar.copy(v_nat[:, t, :], acc[:])

        # ---- q^T (rq folded into the PSUM-drain mul)
        q_nope = [big.tile([128, SEQ], BF16, tag=f"qn{h}", name=f"qn{h}")
                  for h in range(2)]
        qpe_raw = tmp1.tile([128, SEQ], BF16, tag="qpe_raw")
        qswap = tmp1.tile([128, SEQ], BF16, tag="qswap")
        qpe2 = qpe_raw  # rope overwrites the raw rows in place
        qpe_h1 = big.tile([ROPE, SEQ], BF16, tag="qpeh1")

        def q_proj(n, m):
            nsl = slice(n * 512, (n + 1) * 512)
            src_t = tnT_a if n < 2 else tnT_b
            lsl = slice((n % 2) * 512, (n % 2 + 1) * 512)
            acc = psum.tile([128, 512], F32, tag="scores", bufs=3, name="qacc")
            for k in range(KQ):
                nc.tensor.matmul(acc[:], sb_wqb[:, k, m * 128:(m + 1) * 128],
                                 src_t[:, k, lsl],
                                 start=(k == 0), stop=(k == KQ - 1))
            dst = q_nope[m] if m < 2 else qpe_raw
            nc.vector.tensor_mul(dst[:, nsl], acc[:], rq_b[:, nsl])

        def rope_n(n):
            sl = slice(n * 512, (n + 1) * 512)
            sw = psum.tile([128, 512], F32, tag="scores", bufs=3, name="sw")
            nc.tensor.matmul(sw[:], sb_perm2[:], qpe_raw[:, sl],
                             start=True, stop=True)
            nc.vector.tensor_copy(qswap[:, sl], sw[:])
            nc.vector.tensor_mul(qpe2[:, sl], qpe_raw[:, sl], sb_cos2[:, sl])
            nc.vector.tensor_mul(qswap[:, sl], qswap[:, sl], sb_sin2[:, sl])
            nc.vector.tensor_add(qpe2[:, sl], qpe2[:, sl], qswap[:, sl])
            # (qpe2 aliases qpe_raw; the perm matmul consumed the raw values)
            # h1 rope rows to a base-0 tile (partition shift needs DMA)
            nc.sync.dma_start(qpe_h1[:, sl], qpe2[ROPE:128, sl])

        def qpe_of(h):
            return qpe2[0:ROPE, :] if h == 0 else qpe_h1[:, :]

        # lead-in emission: only what r00 (= sb0, h0) needs up front:
        # q m0/m2 + rope for tokens 0:1024, k_nope[0], v. The h1 tiles
        # (q m1, k_nope[1]) and the sb1 q-tiles ride r00/r01 as fillers.
        q_proj(0, 0)
        q_proj(0, 2)
        for n in range(4):
            kn_proj(0, n)
        q_proj(1, 0)
        q_proj(1, 2)
        rope_n(0)
        rope_n(1)
        for t in range(ST):
            v_proj(t)

        f00 = [lambda: q_proj(0, 1), lambda: q_proj(1, 1)]
        f00 += [lambda n=n: kn_proj(1, n) for n in range(4)]
        f00 += [lambda: None, lambda: None]

        f01x = [lambda n=n, m=m: q_proj(n, m) for n in (2, 3) for m in (0, 2, 1)]
        f01x.insert(3, lambda: rope_n(2))
        f01x.append(lambda: rope_n(3))

        # ---- attention: scores^T -> exp pairs -> av (PSUM) + den (DVE tree)
        # av accumulates one 512-query psl at a time (1 bank) so scores get
        # 3 buffers; psl1 replays the kept exp tiles as phase B, interleaved
        # into the next pass. den: DVE pairwise adds + 2 tiny ones-matmuls.
        att_sb = [big.tile([128, 2, SB], BF16, tag=f"att{b}", name=f"att{b}")
                  for b in range(NSB)]

        def av_psl(h, tp, e2, psl_i, av_t):
            psl = slice(psl_i * 512, (psl_i + 1) * 512)
            for half in range(2):
                nc.tensor.matmul(av_t[:],
                                 v_nat[:, 2 * tp + half,
                                       h * VDIM:(h + 1) * VDIM],
                                 e2[:, half, psl],
                                 start=(tp == 0 and half == 0),
                                 stop=(tp == ST // 2 - 1 and half == 1))

        def den_psl(den_acc, psl_i):
            """ones-matmul partition sums of the accumulated exp pairs."""
            psl = slice(psl_i * 512, (psl_i + 1) * 512)
            den_ps = psum.tile([1, 512], F32, tag="den", bufs=1, name="den_ps")
            for half in range(2):
                nc.tensor.matmul(den_ps[:], sb_ones[:], den_acc[:, half, psl],
                                 start=(half == 0), stop=(half == 1))
            return den_ps

        def finish_psl(sb_i, h, psl_i, av_t, den_ps):
            psl = slice(psl_i * 512, (psl_i + 1) * 512)
            den_r = work.tile([1, 512], F32, tag="denr", name="den_r")
            nc.vector.reciprocal(den_r[:], den_ps[:])
            den_b = work.tile([128, 512], F32, tag="denb", bufs=1, name="den_b")
            nc.gpsimd.partition_broadcast(den_b[:], den_r[:])
            nc.vector.tensor_mul(att_sb[sb_i][:, h, psl], av_t[:], den_b[:])

        def attention_pass(sb_i, h, fillers=()):
            fillers = list(fillers)
            s0 = sb_i * SB
            state = {}

            def lazy_av():
                if "av0" not in state:
                    state["av0"] = psum.tile([128, 512], F32, tag="av",
                                             bufs=1, name="av0")
                return state["av0"]

            den_acc = exp_pool.tile([128, 2, SB], BF16, tag="dacc", bufs=1,
                                    name="den_acc")
            e2s = []
            pending = []
            for tp in range(ST // 2):
                # deferred work first so it isn't stuck behind blocked sc allocs
                if len(pending) >= 2:
                    p = pending.pop(0)
                    av_psl(h, p, e2s[p], 0, lazy_av())
                    if p == 0:
                        nc.vector.tensor_add(den_acc[:], e2s[0][:], e2s[1][:])
                    elif p >= 2:
                        nc.vector.tensor_add(den_acc[:], den_acc[:],
                                             e2s[p][:])
                if fillers:
                    fillers.pop(0)()
                e2 = exp_pool.tile([128, 2, SB], BF16, tag="exp2", bufs=8,
                                   name="e2")
                e2s.append(e2)
                for half in range(2):
                    t = 2 * tp + half
                    sc = psum.tile([128, SB], F32, tag="scores", bufs=3,
                                   name="sc")
                    for n2 in range(SB // 512):
                        sl = slice(s0 + n2 * 512, s0 + (n2 + 1) * 512)
                        psl = slice(n2 * 512, (n2 + 1) * 512)
                        nc.tensor.matmul(sc[:, psl],
                                         k_nope[h][:, t * 128:(t + 1) * 128],
                                         q_nope[h][:, sl],
                                         start=True, stop=False)
                        nc.tensor.matmul(sc[:, psl],
                                         sb_kpe[:, t * 128:(t + 1) * 128],
                                         qpe_of(h)[:, sl],
                                         start=False, stop=True)
                    nc.scalar.activation(e2[:, half, :], sc[:], AF.Exp,
                                         scale=SCALE)
                pending.append(tp)
            av0 = lazy_av()
            for p in pending:
                av_psl(h, p, e2s[p], 0, av0)
                if p >= 2:
                    nc.vector.tensor_add(den_acc[:], den_acc[:], e2s[p][:])
            for f in fillers:
                f()
            finish_psl(sb_i, h, 0, av0, den_psl(den_acc, 0))
            return den_acc, e2s

        def phase_b(sb_i, h, state):
            den_acc, e2s = state
            av1 = psum.tile([128, 512], F32, tag="av", bufs=1, name="av1")
            for tp in range(ST // 2):
                av_psl(h, tp, e2s[tp], 1, av1)
            finish_psl(sb_i, h, 1, av1, den_psl(den_acc, 1))

        def oproj_ms(sb_i, ms, engines):
            s0 = sb_i * SB
            att2 = att_sb[sb_i]
            o = opool.tile([128, D_MODEL], BF16, tag="osb", bufs=2, name="o")
            for n in range(D_MODEL // 512):
                acc = psum.tile([128, 512], F32, tag="scores", bufs=3,
                                name="oacc")
                for h in range(2):
                    nc.tensor.matmul(acc[:],
                                     att2[:, h, ms * 128:(ms + 1) * 128],
                                     sb_wo[:, h, n * 512:(n + 1) * 512],
                                     start=(h == 0), stop=(h == 1))
                eng = engines[n % len(engines)]
                if eng is nc.scalar:
                    eng.copy(o[:, n * 512:(n + 1) * 512], acc[:])
                else:
                    eng.tensor_copy(o[:, n * 512:(n + 1) * 512], acc[:])
                if n == 1 or n == 3:  # fire each 1024-half as soon as ready
                    hs = slice((n - 1) * 512, (n + 1) * 512)
                    nc.sync.dma_start(
                        out[s0 + ms * 128: s0 + (ms + 1) * 128, hs],
                        o[:, hs])

        # fillers: prev pass's psl1 replay (phase B) leads each later pass;
        # oproj(0) rides r10/r11
        r00 = attention_pass(0, 0, f00)
        f01 = [lambda: phase_b(0, 0, r00)] + f01x
        r01 = attention_pass(0, 1, f01)

        f10 = [lambda: phase_b(0, 1, r01)]
        f10 += [lambda ms=ms: oproj_ms(0, ms, (nc.vector, nc.scalar))
                for ms in range(4)]
        r10 = attention_pass(1, 0, f10)

        f11 = [lambda: phase_b(1, 0, r10)]
        f11 += [lambda ms=ms: oproj_ms(0, ms, (nc.vector, nc.scalar))
                for ms in range(4, 8)]
        r11 = attention_pass(1, 1, f11)

        # tail: oproj(1) ms0-3 only needs the psl0 halves (cols 0:512), which
        # attention_pass already finished; ACT is free again here
        for ms in range(4):
            oproj_ms(1, ms, (nc.vector, nc.scalar))
        phase_b(1, 1, r11)
        for ms in range(4, 8):
            oproj_ms(1, ms, (nc.vector, nc.scalar))

    nc.compile()
    return nc


# --------------------------------------------------------------------------
# Host orchestration
# --------------------------------------------------------------------------

def _prep(x, freqs_cis, Wqa, qln, Wqb, Wkva, kvln, Wkvb, Wo):
    """Host-side sharding prep (numpy reshapes/casts only)."""
    KD = D_MODEL // 128
    xT = np.ascontiguousarray(x[0].T)                         # [D, S] f32
    xTp = _pack(xT.astype(BF), KD)                            # [128, KD, S]
    cos = freqs_cis[..., 0].astype(np.float32)                # [S, 32]
    sin = freqs_cis[..., 1].astype(np.float32)
    cosT = np.repeat(np.ascontiguousarray(cos.T), 2, axis=0)  # [64, S]
    sinT = np.repeat(np.ascontiguousarray(sin.T), 2, axis=0)

    Wqa_p = _pack(Wqa.astype(BF), KD)
    Wkva_p = _pack(Wkva.astype(BF), KD)

    Wqb_f = (Wqb * qln[:, None]).astype(np.float32)
    Wkvb_f = (Wkvb * kvln[:, None]).astype(np.float32)
    Wqb_hd = Wqb_f.reshape(Q_LORA, NH, QHD)
    Wkvb_hd = Wkvb_f.reshape(KV_LORA, NH, NOPE + VDIM)
    Wo_hd = Wo.reshape(NH, VDIM, D_MODEL)
    l2_per_core = []
    for c in range(N_CORES):
        hs = [2 * c, 2 * c + 1]
        wqb_c = np.concatenate(
            [Wqb_hd[:, hs[0], :NOPE], Wqb_hd[:, hs[1], :NOPE],
             Wqb_hd[:, hs[0], NOPE:], Wqb_hd[:, hs[1], NOPE:]], axis=1)
        wkn_c = np.concatenate([Wkvb_hd[:, h, :NOPE] for h in hs], axis=1)
        wv_c = np.concatenate([Wkvb_hd[:, h, NOPE:] for h in hs], axis=1)
        wo_c = np.concatenate([Wo_hd[h] for h in hs], axis=0)
        l2_per_core.append(dict(
            Wqb=_pack(wqb_c.astype(BF), Q_LORA // 128),
            Wkn=_pack(wkn_c.astype(BF), KV_LORA // 128),
            Wv=_pack(wv_c.astype(BF), KV_LORA // 128),
            Wo=_pack(wo_c.astype(BF), 2),
        ))

    return dict(xTp=xTp, cosT=cosT, sinT=sinT,
                Wqa=Wqa_p, Wkva=Wkva_p,
                ones=np.ones((128, 1), BF),
                perm64=_perm_rope_T(ROPE), perm128=_perm_rope_T(128),
                cosT2=np.concatenate([cosT, cosT], axis=0).astype(BF),
                sinT2=np.concatenate([sinT, sinT], axis=0).astype(BF),
                l2=l2_per_core)


def _get_programs():
    if "l1" not in _CACHE:
        _CACHE["l1"] = build_l1()
    if "l2" not in _CACHE:
        _CACHE["l2"] = build_l2()
    return _CACHE["l1"], _CACHE["l2"]


def kernel(x, mask, freqs_cis, Wqa, qln, Wqb, Wkva, kvln, Wkvb, Wo,
           _trace=False, _tmpdirs=None):
    p = _prep(x, freqs_cis, Wqa, qln, Wqb, Wkva, kvln, Wkvb, Wo)
    l1, l2 = _get_programs()

    in1 = []
    for c in range(N_CORES):
        sl = slice(c * S_LOC, (c + 1) * S_LOC)
        in1.append(dict(
            xT=np.ascontiguousarray(p["xTp"][:, :, sl]),
            Wqa=p["Wqa"], Wkva=p["Wkva"],
            cosT=np.ascontiguousarray(p["cosT"][:, sl]),
            sinT=np.ascontiguousarray(p["sinT"][:, sl]),
            permT=p["perm64"], ones=p["ones"],
        ))
    kw1 = {}
    if _trace:
        kw1 = dict(trace=True, tmpdir=(_tmpdirs or [None, None])[0])
    r1 = run_bass_kernel_spmd(l1, in1, core_ids=list(range(N_CORES)), **kw1)

    tnT = np.concatenate([r1.results[c]["tnT"] for c in range(N_CORES)], axis=2)
    compT = np.concatenate([r1.results[c]["compT"] for c in range(N_CORES)],
                           axis=2)
    kpeT = np.concatenate([r1.results[c]["kpeT"] for c in range(N_CORES)],
                          axis=1)
    sqq = np.concatenate([r1.results[c]["sqq"] for c in range(N_CORES)], axis=1)
    rq_v = 1.0 / np.sqrt(sqq[0].astype(np.float64) / Q_LORA + EPS)
    rq_row = rq_v.astype(np.float32)[None, :]                   # [1, SEQ]

    in2 = []
    for c in range(N_CORES):
        d = dict(tnT=tnT, compT=compT, kpeT=kpeT,
                 cosT2=p["cosT2"], sinT2=p["sinT2"], permT2=p["perm128"],
                 ones=p["ones"], rq=rq_row)
        d.update(p["l2"][c])
        in2.append(d)
    kw2 = {}
    if _trace:
        kw2 = dict(trace=True, tmpdir=(_tmpdirs or [None, None])[1])
    r2 = run_bass_kernel_spmd(l2, in2, core_ids=list(range(N_CORES)), **kw2)

    acc = np.zeros((SEQ, D_MODEL), np.float64)
    for c in range(N_CORES):
        acc += r2.results[c]["out"].astype(np.float64)
    out = acc.astype(np.float32)[None]  # [1, S, D]

    kernel._last = (r1, r2)
    return out
